# revision 1
# baseline (speedup 1.0000x reference)
"""Trainium2 Bass kernel for nn_Conv_8443905704574.

Reference semantics: 7x7 cross-correlation (stride 1, zero pad 3) applied to
the LAST input channel only; the single-channel result is broadcast to all 3
output channels.

Device algorithm: banded-Toeplitz matmul conv. For each 128-row input block,
the 7 kernel columns become 7 stationary [128,128] band matrices (entries
T[k,m] = K[k-m+off, dj]); each is matmul'd (fp32r, full PE rate) against a
W-shifted slice of the block, accumulating the 7 taps in PSUM. One block
yields 122 valid output rows. The W zero-padding is baked into the host-side
input layout so each input block needs exactly one DMA (keeps per-matmul sync
wait counts within the ISA limit).

Sharding: pure data parallel — 2 images per core across 8 cores; host slices
the last channel, device computes [2,1024,1024], host broadcasts channels.
"""

import numpy as np

import concourse.bacc as bacc
import concourse.mybir as mybir
import concourse.tile as tile
from concourse.bass_utils import run_bass_kernel_spmd

B, C, H, W = 16, 3, 1024, 1024
KS = 7
PAD = KS // 2
NCORES = 8
PER = B // NCORES          # images per core
TILE_OUT = 128 - (KS - 1)  # 122 valid output rows per H-tile
NT = (H + TILE_OUT - 1) // TILE_OUT  # 9
WCH = 512                  # W chunk = one fp32 PSUM bank
NWCH = W // WCH            # 2
XW = W + 2 * PAD           # host-padded input width
NXB = 5                    # input block buffers

f32 = mybir.dt.float32
f32r = mybir.dt.float32r

_CACHE = {}
LAST_RESULTS = None


def _build_bass():
    nc = bacc.Bacc("TRN2", target_bir_lowering=False, debug=False)
    x = nc.dram_tensor("x", [PER, H, XW], f32r, kind="ExternalInput")
    tmat = nc.dram_tensor("tmat", [128, 2 * KS * 128], f32r, kind="ExternalInput")
    out = nc.dram_tensor("out", [PER, H, W], f32, kind="ExternalOutput")

    with tile.TileContext(nc) as tc:
        with (
            tc.tile_pool(name="tmp", bufs=1) as tmpool,
            tc.tile_pool(name="xp", bufs=1) as xpool,
            tc.tile_pool(name="op", bufs=6) as opool,
            tc.tile_pool(name="pp", bufs=6, space="PSUM") as ppool,
            tc.tile_pool(name="wz", bufs=1) as wzpool,
        ):
            x_tiles = []
            for i in range(NXB):
                xt = xpool.tile([128, XW], f32r, name=f"xt{i}", tag=f"xt{i}")
                x_tiles.append(xt)

            # PE warm-up during the DMA lead-in: zero matmuls keep the PE HAM
            # clock gate busy so real matmuls start at full clock.
            wz = wzpool.tile([128, 128 + WCH], f32, name="wz")
            nc.vector.memset(wz[:], 0.0)
            pz = [
                ppool.tile([128, WCH], f32, name=f"pz{i}", tag=f"pz{i}", bufs=1)
                for i in range(2)
            ]
            for i in range(16):
                nc.tensor.matmul(
                    pz[i % 2][:],
                    wz[:, 0:128].bitcast(f32r),
                    wz[:, 128 : 128 + WCH].bitcast(f32r),
                    start=True, stop=True,
                )

            # first input block + band matrices: the critical path
            t_sb = tmpool.tile([128, 2 * KS * 128], f32r, name="t_sb")

            def tile_geo(img, t):
                r0 = t * TILE_OUT
                nv = min(TILE_OUT, H - r0)
                # First block starts at the image edge (band offset PAD);
                # interior blocks start PAD rows above their outputs.
                if t == 0:
                    in0, variant = 0, 0
                else:
                    in0, variant = r0 - PAD, 1
                nk = min(128, H - in0)
                return r0, nv, in0, nk, variant

            schedule = [(img, t) for img in range(PER) for t in range(NT)]

            # x0 ahead of the band matrices (both gate the first real matmul)
            nc.sync.dma_start(x_tiles[0][0:128, :], x[0, 0:128, :])
            nc.sync.dma_start(t_sb[:], tmat[:])

            for idx, (img, t) in enumerate(schedule):
                r0, nv, in0, nk, variant = tile_geo(img, t)
                xt = x_tiles[idx % NXB]
                if idx > 0:
                    nc.sync.dma_start(xt[0:nk, :], x[img, in0 : in0 + nk, :])
                for c in range(NWCH):
                    pt = ppool.tile([128, WCH], f32, name="pt", tag="pt")
                    for dj in range(KS):
                        col = (variant * KS + dj) * 128
                        nc.tensor.matmul(
                            pt[:],
                            t_sb[0:nk, col : col + 128],
                            xt[0:nk, c * WCH + dj : c * WCH + dj + WCH],
                            start=(dj == 0),
                            stop=(dj == KS - 1),
                        )
                    ot = opool.tile([128, WCH], f32, name="ot", tag="ot")
                    nc.scalar.copy(ot[0:nv, :], pt[0:nv, :])
                    nc.sync.dma_start(
                        out[img, r0 : r0 + nv, c * WCH : (c + 1) * WCH],
                        ot[0:nv, :],
                    )
    nc.compile()
    return nc


def _toeplitz(kmat: np.ndarray) -> np.ndarray:
    """[128, 2*KS*128] stationary band matrices: variant 0 = first block
    (band offset PAD), variant 1 = interior blocks (band offset 0)."""
    k_idx = np.arange(128)[:, None]
    m_idx = np.arange(128)[None, :]
    t_all = np.zeros((128, 2, KS, 128), dtype=np.float32)
    for variant, off in ((0, PAD), (1, 0)):
        di = k_idx - m_idx + off
        mask = (di >= 0) & (di < KS)
        dic = np.clip(di, 0, KS - 1)
        for dj in range(KS):
            t_all[:, variant, dj, :] = np.where(mask, kmat[dic, dj], 0.0)
    return t_all.reshape(128, 2 * KS * 128)


def _shard_inputs(image: np.ndarray, kmat: np.ndarray):
    tmat = _toeplitz(kmat)
    xs = np.zeros((NCORES, PER, H, XW), dtype=np.float32)
    xs[:, :, :, PAD : PAD + W] = image[:, C - 1, :, :].reshape(
        NCORES, PER, H, W
    )
    return [{"x": xs[i], "tmat": tmat} for i in range(NCORES)]


def kernel(**inputs):
    global LAST_RESULTS
    image = np.asarray(inputs["image"], dtype=np.float32)
    kmat = np.asarray(inputs["kernel"], dtype=np.float32)
    assert image.shape == (B, C, H, W), image.shape

    if "nc" not in _CACHE:
        _CACHE["nc"] = _build_bass()
    nc = _CACHE["nc"]

    in_maps = _shard_inputs(image, kmat)
    res = run_bass_kernel_spmd(nc, in_maps, list(range(NCORES)))
    LAST_RESULTS = res

    y = np.stack([res.results[i]["out"] for i in range(NCORES)], axis=0)
    y = y.reshape(B, 1, H, W)
    return np.broadcast_to(y, (B, C, H, W))



# revision 3
# speedup vs baseline: 27542.1494x; 27542.1494x over previous
"""Trainium2 Bass kernel for nn_Conv_8443905704574.

Reference semantics: 7x7 cross-correlation (stride 1, zero pad 3) applied to
the LAST input channel only; the single-channel result is broadcast to all 3
output channels.

Device algorithm: banded-Toeplitz matmul conv. For each 128-row input block,
the 7 kernel columns become 7 stationary [128,128] band matrices (entries
T[k,m] = K[k-m+off, dj]); each is matmul'd (fp32r, full PE rate) against a
W-shifted slice of the block, accumulating the 7 taps in PSUM. One block
yields 122 valid output rows. The W zero-padding is baked into the host-side
input layout so each input block needs exactly one DMA (keeps per-matmul sync
wait counts within the ISA limit).

Sharding: pure data parallel — 2 images per core across 8 cores; host slices
the last channel, device computes [2,1024,1024], host broadcasts channels.
"""

import numpy as np

import concourse.bacc as bacc
import concourse.mybir as mybir
import concourse.tile as tile
from concourse.bass_utils import run_bass_kernel_spmd

B, C, H, W = 16, 3, 1024, 1024
KS = 7
PAD = KS // 2
NCORES = 8
PER = B // NCORES          # images per core
TILE_OUT = 128 - (KS - 1)  # 122 valid output rows per H-tile
NT = (H + TILE_OUT - 1) // TILE_OUT  # 9
WCH = 512                  # W chunk = one fp32 PSUM bank
NWCH = W // WCH            # 2
XW = W + 2 * PAD           # host-padded input width
NXB = 5                    # input block buffers

f32 = mybir.dt.float32
f32r = mybir.dt.float32r

_CACHE = {}
LAST_RESULTS = None


def _build_bass(reps: int = 1):
    nc = bacc.Bacc("TRN2", target_bir_lowering=False, debug=False)
    x = nc.dram_tensor("x", [PER, H, XW], f32r, kind="ExternalInput")
    tmat = nc.dram_tensor("tmat", [128, 2 * KS * 128], f32r, kind="ExternalInput")
    out = nc.dram_tensor("out", [PER, H, W], f32, kind="ExternalOutput")

    with tile.TileContext(nc) as tc:
        with (
            tc.tile_pool(name="tmp", bufs=1) as tmpool,
            tc.tile_pool(name="xp", bufs=1) as xpool,
            tc.tile_pool(name="op", bufs=6) as opool,
            tc.tile_pool(name="pp", bufs=6, space="PSUM") as ppool,
            tc.tile_pool(name="wz", bufs=1) as wzpool,
        ):
            x_tiles = []
            for i in range(NXB):
                xt = xpool.tile([128, XW], f32r, name=f"xt{i}", tag=f"xt{i}")
                x_tiles.append(xt)

            # PE warm-up during the DMA lead-in: zero matmuls keep the PE HAM
            # clock gate busy so real matmuls start at full clock.
            wz = wzpool.tile([128, 128 + WCH], f32, name="wz")
            nc.vector.memset(wz[:], 0.0)
            pz = [
                ppool.tile([128, WCH], f32, name=f"pz{i}", tag=f"pz{i}", bufs=1)
                for i in range(2)
            ]
            for i in range(16):
                nc.tensor.matmul(
                    pz[i % 2][:],
                    wz[:, 0:128].bitcast(f32r),
                    wz[:, 128 : 128 + WCH].bitcast(f32r),
                    start=True, stop=True,
                )

            # first input block + band matrices: the critical path
            t_sb = tmpool.tile([128, 2 * KS * 128], f32r, name="t_sb")

            def tile_geo(img, t):
                r0 = t * TILE_OUT
                nv = min(TILE_OUT, H - r0)
                # First block starts at the image edge (band offset PAD);
                # interior blocks start PAD rows above their outputs.
                if t == 0:
                    in0, variant = 0, 0
                else:
                    in0, variant = r0 - PAD, 1
                nk = min(128, H - in0)
                return r0, nv, in0, nk, variant

            schedule = [(img, t) for img in range(PER) for t in range(NT)]

            # x0 ahead of the band matrices (both gate the first real matmul)
            nc.sync.dma_start(x_tiles[0][0:128, :], x[0, 0:128, :])
            nc.sync.dma_start(t_sb[:], tmat[:])

            for rep in range(reps):
              for idx, (img, t) in enumerate(schedule):
                r0, nv, in0, nk, variant = tile_geo(img, t)
                xt = x_tiles[idx % NXB]
                if idx > 0 or rep > 0:
                    nc.sync.dma_start(xt[0:nk, :], x[img, in0 : in0 + nk, :])
                for c in range(NWCH):
                    pt = ppool.tile([128, WCH], f32, name="pt", tag="pt")
                    for dj in range(KS):
                        col = (variant * KS + dj) * 128
                        nc.tensor.matmul(
                            pt[:],
                            t_sb[0:nk, col : col + 128],
                            xt[0:nk, c * WCH + dj : c * WCH + dj + WCH],
                            start=(dj == 0),
                            stop=(dj == KS - 1),
                        )
                    ot = opool.tile([128, WCH], f32, name="ot", tag="ot")
                    nc.scalar.copy(ot[0:nv, :], pt[0:nv, :])
                    nc.sync.dma_start(
                        out[img, r0 : r0 + nv, c * WCH : (c + 1) * WCH],
                        ot[0:nv, :],
                    )
    nc.compile()
    return nc


def _toeplitz(kmat: np.ndarray) -> np.ndarray:
    """[128, 2*KS*128] stationary band matrices: variant 0 = first block
    (band offset PAD), variant 1 = interior blocks (band offset 0)."""
    k_idx = np.arange(128)[:, None]
    m_idx = np.arange(128)[None, :]
    t_all = np.zeros((128, 2, KS, 128), dtype=np.float32)
    for variant, off in ((0, PAD), (1, 0)):
        di = k_idx - m_idx + off
        mask = (di >= 0) & (di < KS)
        dic = np.clip(di, 0, KS - 1)
        for dj in range(KS):
            t_all[:, variant, dj, :] = np.where(mask, kmat[dic, dj], 0.0)
    return t_all.reshape(128, 2 * KS * 128)


def _shard_inputs(image: np.ndarray, kmat: np.ndarray):
    tmat = _toeplitz(kmat)
    xs = np.zeros((NCORES, PER, H, XW), dtype=np.float32)
    xs[:, :, :, PAD : PAD + W] = image[:, C - 1, :, :].reshape(
        NCORES, PER, H, W
    )
    return [{"x": xs[i], "tmat": tmat} for i in range(NCORES)]


def kernel(**inputs):
    global LAST_RESULTS
    image = np.asarray(inputs["image"], dtype=np.float32)
    kmat = np.asarray(inputs["kernel"], dtype=np.float32)
    assert image.shape == (B, C, H, W), image.shape

    if "nc" not in _CACHE:
        _CACHE["nc"] = _build_bass()
    nc = _CACHE["nc"]

    in_maps = _shard_inputs(image, kmat)
    res = run_bass_kernel_spmd(nc, in_maps, list(range(NCORES)))
    LAST_RESULTS = res

    y = np.stack([res.results[i]["out"] for i in range(NCORES)], axis=0)
    y = y.reshape(B, 1, H, W)
    return np.broadcast_to(y, (B, C, H, W))



# revision 4
# speedup vs baseline: 82821.8362x; 3.0071x over previous
"""Trainium2 Bass kernel for nn_Conv_8443905704574.

Reference semantics: 7x7 cross-correlation (stride 1, zero pad 3) applied to
the LAST input channel only; the single-channel result is broadcast to all 3
output channels.

Device algorithm: banded-Toeplitz matmul conv in bf16. For each 128-row input
block, the 7 kernel columns become 7 stationary [128,128] band matrices
(entries T[k,m] = K[k-m+off, dj]); each is matmul'd against a W-shifted slice
of the block, accumulating the 7 taps in fp32 PSUM. One block yields 122
valid output rows. The core's two images are concatenated along W (with the
zero padding baked in on the host) so each block is ONE wide DMA (4120B
lines) and the four 512-col PSUM chunks are copied (with bf16 cast) into one
SBUF tile flushed by ONE output DMA (4096B lines).

Sharding: pure data parallel - 2 images per core across 8 cores; host slices
the last channel, casts to bf16, device computes [1024, 2048] (two images
side by side), host upcasts and broadcasts channels.
"""

import numpy as np
import ml_dtypes

import concourse.bacc as bacc
import concourse.mybir as mybir
import concourse.tile as tile
from concourse.bass_utils import run_bass_kernel_spmd

B, C, H, W = 16, 3, 1024, 1024
KS = 7
PAD = KS // 2
NCORES = 8
PER = B // NCORES          # images per core
TILE_OUT = 128 - (KS - 1)  # 122 valid output rows per H-tile
NT = (H + TILE_OUT - 1) // TILE_OUT  # 9
WCH = 512                  # W chunk = one fp32 PSUM bank
XW = W + 2 * PAD           # per-image padded input width
CW = PER * XW              # concat input width (2 images)
OW = PER * W               # concat output width
NCH = PER * (W // WCH)     # 4 PSUM chunks per block
NXB = 4                    # input block buffers

f32 = mybir.dt.float32
bf16 = mybir.dt.bfloat16
np_bf16 = ml_dtypes.bfloat16

_CACHE = {}
LAST_RESULTS = None


def _tile_geo(t):
    r0 = t * TILE_OUT
    nv = min(TILE_OUT, H - r0)
    # First block starts at the image edge (band offset PAD); interior
    # blocks start PAD rows above their outputs.
    if t == 0:
        in0, variant = 0, 0
    else:
        in0, variant = r0 - PAD, 1
    nk = min(128, H - in0)
    return r0, nv, in0, nk, variant


def _build_bass(reps: int = 1):
    nc = bacc.Bacc("TRN2", target_bir_lowering=False, debug=False)
    x = nc.dram_tensor("x", [H, CW], bf16, kind="ExternalInput")
    tmat = nc.dram_tensor("tmat", [128, 2 * KS * 128], bf16, kind="ExternalInput")
    out = nc.dram_tensor("out", [H, OW], bf16, kind="ExternalOutput")

    with tile.TileContext(nc) as tc:
        with (
            tc.tile_pool(name="tmp", bufs=1) as tmpool,
            tc.tile_pool(name="xp", bufs=1) as xpool,
            tc.tile_pool(name="op", bufs=3) as opool,
            tc.tile_pool(name="pp", bufs=6, space="PSUM") as ppool,
            tc.tile_pool(name="wz", bufs=1) as wzpool,
        ):
            x_tiles = []
            for i in range(NXB):
                xt = xpool.tile([128, CW], bf16, name=f"xt{i}", tag=f"xt{i}")
                x_tiles.append(xt)

            # PE warm-up during the DMA lead-in: zero matmuls keep the PE HAM
            # clock gate busy so real matmuls start at full clock.
            wz = wzpool.tile([128, 128 + WCH], bf16, name="wz")
            nc.vector.memset(wz[:], 0.0)
            pz = [
                ppool.tile([128, WCH], f32, name=f"pz{i}", tag=f"pz{i}", bufs=1)
                for i in range(2)
            ]
            for i in range(16):
                nc.tensor.matmul(
                    pz[i % 2][:],
                    wz[:, 0:128],
                    wz[:, 128 : 128 + WCH],
                    start=True, stop=True,
                )

            # first input block + band matrices: the critical path
            t_sb = tmpool.tile([128, 2 * KS * 128], bf16, name="t_sb")
            nc.sync.dma_start(x_tiles[0][0:128, :], x[0:128, :])
            nc.sync.dma_start(t_sb[:], tmat[:])

            for rep in range(reps):
              for t in range(NT):
                r0, nv, in0, nk, variant = _tile_geo(t)
                xt = x_tiles[t % NXB]
                if t > 0 or rep > 0:
                    nc.sync.dma_start(xt[0:nk, :], x[in0 : in0 + nk, :])
                ot = opool.tile([128, OW], bf16, name="ot", tag="ot")
                for ch in range(NCH):
                    img, hc = divmod(ch, W // WCH)
                    xbase = img * XW + hc * WCH
                    pt = ppool.tile([128, WCH], f32, name="pt", tag="pt")
                    for dj in range(KS):
                        col = (variant * KS + dj) * 128
                        nc.tensor.matmul(
                            pt[:],
                            t_sb[0:nk, col : col + 128],
                            xt[0:nk, xbase + dj : xbase + dj + WCH],
                            start=(dj == 0),
                            stop=(dj == KS - 1),
                        )
                    nc.scalar.copy(
                        ot[0:nv, ch * WCH : (ch + 1) * WCH], pt[0:nv, :]
                    )
                nc.sync.dma_start(out[r0 : r0 + nv, :], ot[0:nv, :])
    nc.compile()
    return nc


def _toeplitz(kmat: np.ndarray) -> np.ndarray:
    """[128, 2*KS*128] stationary band matrices: variant 0 = first block
    (band offset PAD), variant 1 = interior blocks (band offset 0)."""
    k_idx = np.arange(128)[:, None]
    m_idx = np.arange(128)[None, :]
    t_all = np.zeros((128, 2, KS, 128), dtype=np.float32)
    for variant, off in ((0, PAD), (1, 0)):
        di = k_idx - m_idx + off
        mask = (di >= 0) & (di < KS)
        dic = np.clip(di, 0, KS - 1)
        for dj in range(KS):
            t_all[:, variant, dj, :] = np.where(mask, kmat[dic, dj], 0.0)
    return t_all.reshape(128, 2 * KS * 128).astype(np_bf16)


def _shard_inputs(image: np.ndarray, kmat: np.ndarray):
    tmat = _toeplitz(kmat)
    last = image[:, C - 1, :, :].astype(np_bf16)  # [B, H, W]
    xs = np.zeros((NCORES, H, CW), dtype=np_bf16)
    for p in range(PER):
        xs[:, :, p * XW + PAD : p * XW + PAD + W] = last[p::PER]
    return [{"x": xs[i], "tmat": tmat} for i in range(NCORES)]


def kernel(**inputs):
    global LAST_RESULTS
    image = np.asarray(inputs["image"], dtype=np.float32)
    kmat = np.asarray(inputs["kernel"], dtype=np.float32)
    assert image.shape == (B, C, H, W), image.shape

    if "nc" not in _CACHE:
        _CACHE["nc"] = _build_bass()
    nc = _CACHE["nc"]

    in_maps = _shard_inputs(image, kmat)
    res = run_bass_kernel_spmd(nc, in_maps, list(range(NCORES)))
    LAST_RESULTS = res

    y = np.empty((B, 1, H, W), dtype=np.float32)
    for i in range(NCORES):
        o = res.results[i]["out"]  # [H, 2*W] bf16
        for p in range(PER):
            y[PER * i + p, 0] = o[:, p * W : (p + 1) * W].astype(np.float32)
    return np.broadcast_to(y, (B, C, H, W))


# revision 6
# speedup vs baseline: 491257.4364x; 5.9315x over previous
"""Trainium2 Bass kernel for nn_Conv_8443905704574.

Reference semantics: 7x7 cross-correlation (stride 1, zero pad 3) applied to
the LAST input channel only; the single-channel result is broadcast to all 3
output channels.

Device algorithm: banded-Toeplitz matmul conv in bf16. For each 128-row input
block, the 7 kernel columns become 7 stationary [128,128] band matrices
(entries T[k,m] = K[k-m+off, dj]); each is matmul'd against a W-shifted slice
of the block, accumulating the 7 taps in fp32 PSUM. One block yields 122
valid output rows. The core's two images are concatenated along W (with the
zero padding baked in on the host) so each block is ONE wide DMA (4120B
lines) and the four 512-col PSUM chunks are copied (with bf16 cast) into one
SBUF tile flushed by ONE output DMA (4096B lines).

Sharding: pure data parallel - 2 images per core across 8 cores; host slices
the last channel, casts to bf16, device computes [1024, 2048] (two images
side by side), host upcasts and broadcasts channels.
"""

import numpy as np
import ml_dtypes

import concourse.bacc as bacc
import concourse.mybir as mybir
import concourse.tile as tile
from concourse.bass_utils import run_bass_kernel_spmd

B, C, H, W = 16, 3, 1024, 1024
KS = 7
PAD = KS // 2
NCORES = 8
PER = B // NCORES          # images per core
TILE_OUT = 128 - (KS - 1)  # 122 valid output rows per H-tile
NT = (H + TILE_OUT - 1) // TILE_OUT  # 9
WCH = 512                  # W chunk = one fp32 PSUM bank
XW = W + 2 * PAD           # per-image padded input width
CW = PER * XW              # concat input width (2 images)
OW = PER * W               # concat output width
NCH = PER * (W // WCH)     # 4 PSUM chunks per block
NXB = 4                    # input block buffers

f32 = mybir.dt.float32
bf16 = mybir.dt.bfloat16
np_bf16 = ml_dtypes.bfloat16

_CACHE = {}
LAST_RESULTS = None


def _tile_geo(t):
    r0 = t * TILE_OUT
    nv = min(TILE_OUT, H - r0)
    # First block starts at the image edge (band offset PAD); interior
    # blocks start PAD rows above their outputs.
    if t == 0:
        in0, variant = 0, 0
    else:
        in0, variant = r0 - PAD, 1
    nk = min(128, H - in0)
    return r0, nv, in0, nk, variant


def _build_bass(
    reps: int = 1,
    do_in_dma: bool = True,
    do_mm: bool = True,
    do_out_dma: bool = True,
    dj_outer: bool = True,
    act_out: bool = True,
):
    nc = bacc.Bacc("TRN2", target_bir_lowering=False, debug=False)
    x = nc.dram_tensor("x", [H, CW], bf16, kind="ExternalInput")
    tmat = nc.dram_tensor("tmat", [128, 2 * KS * 128], bf16, kind="ExternalInput")
    out = nc.dram_tensor("out", [H, OW], bf16, kind="ExternalOutput")

    with tile.TileContext(nc) as tc:
        with (
            tc.tile_pool(name="tmp", bufs=1) as tmpool,
            tc.tile_pool(name="xp", bufs=1) as xpool,
            tc.tile_pool(name="op", bufs=3) as opool,
            tc.tile_pool(name="pp", bufs=8, space="PSUM") as ppool,
            tc.tile_pool(name="wz", bufs=1) as wzpool,
        ):
            x_tiles = []
            for i in range(NXB):
                xt = xpool.tile([128, CW], bf16, name=f"xt{i}", tag=f"xt{i}")
                x_tiles.append(xt)
            if not do_in_dma:
                for xt in x_tiles:
                    nc.vector.memset(xt[:], 0.0)

            # PE warm-up during the DMA lead-in: zero matmuls keep the PE HAM
            # clock gate busy so real matmuls start at full clock.
            if do_mm:
                wz = wzpool.tile([128, 128 + WCH], bf16, name="wz")
                nc.vector.memset(wz[:], 0.0)
                for i in range(16):
                    pz = ppool.tile([128, WCH], f32, name="pz", tag="pt")
                    nc.tensor.matmul(
                        pz[:],
                        wz[:, 0:128],
                        wz[:, 128 : 128 + WCH],
                        start=True, stop=True,
                    )

            # first input block + band matrices: the critical path
            t_sb = tmpool.tile([128, 2 * KS * 128], bf16, name="t_sb")
            if do_in_dma:
                nc.sync.dma_start(x_tiles[0][0:128, :], x[0:128, :])
            nc.sync.dma_start(t_sb[:], tmat[:])

            out_eng = nc.scalar if act_out else nc.sync

            for rep in range(reps):
              for t in range(NT):
                r0, nv, in0, nk, variant = _tile_geo(t)
                xt = x_tiles[t % NXB]
                if do_in_dma and (t > 0 or rep > 0):
                    nc.sync.dma_start(xt[0:nk, :], x[in0 : in0 + nk, :])
                ot = opool.tile([128, OW], bf16, name="ot", tag="ot")
                if do_mm:
                    pts = [
                        ppool.tile([128, WCH], f32, name=f"pt{ch}", tag="pt")
                        for ch in range(NCH)
                    ]
                    if dj_outer:
                        order = [
                            (dj, ch) for dj in range(KS) for ch in range(NCH)
                        ]
                    else:
                        order = [
                            (dj, ch) for ch in range(NCH) for dj in range(KS)
                        ]
                    for dj, ch in order:
                        img, hc = divmod(ch, W // WCH)
                        xbase = img * XW + hc * WCH
                        col = (variant * KS + dj) * 128
                        nc.tensor.matmul(
                            pts[ch][:],
                            t_sb[0:nk, col : col + 128],
                            xt[0:nk, xbase + dj : xbase + dj + WCH],
                            start=(dj == 0),
                            stop=(dj == KS - 1),
                        )
                    for ch in range(NCH):
                        nc.scalar.copy(
                            ot[0:nv, ch * WCH : (ch + 1) * WCH], pts[ch][0:nv, :]
                        )
                elif rep == 0 and t < 3:
                    nc.vector.memset(ot[:], 0.0)
                if do_out_dma:
                    out_eng.dma_start(out[r0 : r0 + nv, :], ot[0:nv, :])
    nc.compile()
    return nc


def _toeplitz(kmat: np.ndarray) -> np.ndarray:
    """[128, 2*KS*128] stationary band matrices: variant 0 = first block
    (band offset PAD), variant 1 = interior blocks (band offset 0)."""
    k_idx = np.arange(128)[:, None]
    m_idx = np.arange(128)[None, :]
    t_all = np.zeros((128, 2, KS, 128), dtype=np.float32)
    for variant, off in ((0, PAD), (1, 0)):
        di = k_idx - m_idx + off
        mask = (di >= 0) & (di < KS)
        dic = np.clip(di, 0, KS - 1)
        for dj in range(KS):
            t_all[:, variant, dj, :] = np.where(mask, kmat[dic, dj], 0.0)
    return t_all.reshape(128, 2 * KS * 128).astype(np_bf16)


def _shard_inputs(image: np.ndarray, kmat: np.ndarray):
    tmat = _toeplitz(kmat)
    last = image[:, C - 1, :, :].astype(np_bf16)  # [B, H, W]
    xs = np.zeros((NCORES, H, CW), dtype=np_bf16)
    for p in range(PER):
        xs[:, :, p * XW + PAD : p * XW + PAD + W] = last[p::PER]
    return [{"x": xs[i], "tmat": tmat} for i in range(NCORES)]


def kernel(**inputs):
    global LAST_RESULTS
    image = np.asarray(inputs["image"], dtype=np.float32)
    kmat = np.asarray(inputs["kernel"], dtype=np.float32)
    assert image.shape == (B, C, H, W), image.shape

    if "nc" not in _CACHE:
        _CACHE["nc"] = _build_bass()
    nc = _CACHE["nc"]

    in_maps = _shard_inputs(image, kmat)
    res = run_bass_kernel_spmd(nc, in_maps, list(range(NCORES)))
    LAST_RESULTS = res

    y = np.empty((B, 1, H, W), dtype=np.float32)
    for i in range(NCORES):
        o = res.results[i]["out"]  # [H, 2*W] bf16
        for p in range(PER):
            y[PER * i + p, 0] = o[:, p * W : (p + 1) * W].astype(np.float32)
    return np.broadcast_to(y, (B, C, H, W))
